# revision 15
# baseline (speedup 1.0000x reference)
"""Causal self-attention (B=4, T=2048, C=1024, 16 heads) on 8 NeuronCores.

Sharding: core c -> batch b=c//2, head group g=c%2 (8 heads each).
Each core computes q,k,v for its 8 heads, causal flash-style attention,
and a partial output projection (row-slice of w_proj). Host sums the two
partials per batch and adds b_proj.

v4: all-bf16 pipeline (fp32 matmul is half PE rate; fp8 fails the 2e-2
error gate on logit-noise tails), reciprocal_approx_fast for the softmax
divide, diagonal-refined S/exp/PV windows (only columns right of the
diagonal are computed on diagonal tiles), split projection so the first
half (head pairs 0-1) fills PE gaps during the second half's attention,
column-chunked xT loads so the V phase starts early.

Layout strategy:
  - x is pre-transposed on host -> xT [C, T] so the C-contraction sits on
    SBUF partitions for the qkv projections.
  - q, k are produced transposed (qT/kT [head_dim, T]); the two heads of a
    pair live on partitions 0-63 / 64-127 so their S^T matmuls run
    concurrently in separate PE row groups (tile_position auto-derived).
  - v is produced in natural [T, 512] layout, stored interleaved per-head
    with a ones column (66-stride for 4B alignment): the PV matmul's row 64
    accumulates the softmax denominators for free.
  - Normalization: sums row -> K=1 ones matmul broadcast -> approx
    reciprocal -> multiply into y^T.
"""
import numpy as np
import ml_dtypes
import concourse.bass as bass
from concourse import bacc
import concourse.tile as tile
import concourse.mybir as mybir
from concourse.bass_utils import run_bass_kernel_spmd

B, T, C = 4, 2048, 1024
HD = 64            # head dim
HL = 8             # local heads per core
PAIRS = 4          # local head pairs
KT = C // 128      # 8 contraction tiles for qkv
TT = T // 128      # 16 row tiles of T
NQ = T // 512      # 4 query chunks of 512
F32 = mybir.dt.float32
BF16 = mybir.dt.bfloat16
EXP = mybir.ActivationFunctionType.Exp
VS = 66            # per-head stride in vaug (64 v + 1 ones + 1 pad)

_NC_CACHE = {}


def _build(nrep=1, bias=False):
    nc = bacc.Bacc("TRN2", target_bir_lowering=False, debug=False)
    xT_d = nc.dram_tensor("xT", [C, T], BF16, kind="ExternalInput")
    wqkv_d = nc.dram_tensor("wqkv", [C, 1536], BF16, kind="ExternalInput")
    bqkv_d = nc.dram_tensor("bqkv", [1536], BF16, kind="ExternalInput")
    wp_d = nc.dram_tensor("wp", [512, C], BF16, kind="ExternalInput")
    out_d = nc.dram_tensor("out", [T, C], BF16, kind="ExternalOutput")

    # constants baked into the NEFF
    tri_np = np.zeros((128, 128), dtype=np.float32)
    for p in range(128):
        tri_np[p, p:] = 1.0
    tri_d = nc.inline_tensor(tri_np.astype(ml_dtypes.bfloat16), name="ctri")
    ones_r_d = nc.inline_tensor(
        np.ones((1, 128), dtype=ml_dtypes.bfloat16), name="ones_r")
    ones_v_d = nc.inline_tensor(
        np.ones((128, 8), dtype=ml_dtypes.bfloat16), name="ones_v")

    with tile.TileContext(nc) as tc:
        with (
            tc.tile_pool(name="xt", bufs=8) as p_xt,        # xT tiles
            tc.tile_pool(name="w", bufs=16) as p_w,         # w_v + per-pair w_q/w_k
            tc.tile_pool(name="wp", bufs=4) as p_wp,
            tc.tile_pool(name="vaug", bufs=16) as p_va,
            tc.tile_pool(name="qk", bufs=4) as p_qk,
            tc.tile_pool(name="yt", bufs=4) as p_yt,
            tc.tile_pool(name="mask", bufs=1) as p_mask,
            tc.tile_pool(name="pexp", bufs=4) as p_px,
            tc.tile_pool(name="srow", bufs=3) as p_sr,
            tc.tile_pool(name="rcp", bufs=3) as p_rc,
            tc.tile_pool(name="obh", bufs=32) as p_oh,      # split-proj A half
            tc.tile_pool(name="ob", bufs=3) as p_ob,
            tc.tile_pool(name="tiny", bufs=2) as p_tiny,
            tc.tile_pool(name="mm", bufs=2, space="PSUM") as pp_mm,
            tc.tile_pool(name="st", bufs=2, space="PSUM") as pp_st,
            tc.tile_pool(name="ot", bufs=2, space="PSUM") as pp_ot,
        ):
            # ---- constants ----
            ones_r = p_tiny.tile([1, 128], BF16, tag="onesr")
            nc.sync.dma_start(out=ones_r[:], in_=ones_r_d.ap())
            # ones row living at partition 64 (for K-dim alignment with sums)
            ones64 = p_tiny.tile([65, 128], BF16, tag="ones64")
            nc.sync.dma_start(out=ones64[64:65, :], in_=ones_r_d.ap())
            tri = p_mask.tile([128, 128], BF16, tag="mask")
            nc.sync.dma_start(out=tri[:], in_=tri_d.ap())

            def emit_attention(hp, qt, kt, yt, vaugs, tri, ones64):
                for qb in range(NQ):
                    qsl = slice(qb * 512, (qb + 1) * 512)
                    ots = [pp_ot.tile([65, 512], F32, tag="ot",
                                      name=f"ot{hp}_{qb}_{i}")
                           for i in range(2)]
                    ntk = 4 * qb + 4
                    for tk in range(ntk):
                        ksl = slice(tk * 128, (tk + 1) * 128)
                        diag_j = tk - 4 * qb
                        # diagonal tiles: only columns >= the diagonal are
                        # live; compute the [qoff:512] window only
                        qoff = 0 if diag_j < 0 else 128 * diag_j
                        w = 512 - qoff
                        # one double-bank psum holds both heads' S tiles;
                        # the two matmuls run in separate PE row groups
                        st = pp_st.tile([128, 1024], F32, tag="st")
                        for h01 in range(2):
                            prt = slice(64 * h01, 64 * h01 + 64)
                            nc.tensor.matmul(
                                st[:, 512 * h01:512 * h01 + w],
                                kt[prt, ksl],
                                qt[prt, qb * 512 + qoff:(qb + 1) * 512],
                                start=True, stop=True)
                        px = p_px.tile([128, 1024], BF16, tag="pexp")
                        stv = st[:].rearrange("p (r f) -> p r f", r=2)
                        pxv = px[:].rearrange("p (r f) -> p r f", r=2)
                        nc.scalar.activation(pxv[:, :, 0:w], stv[:, :, 0:w],
                                             EXP, scale=0.125)
                        if diag_j >= 0:
                            # triangular boundary is the first 128 columns
                            # of the window
                            m2 = tri[:].unsqueeze(1).broadcast_to([128, 2, 128])
                            nc.vector.tensor_mul(pxv[:, :, 0:128],
                                                 pxv[:, :, 0:128], m2)
                        for h01 in range(2):
                            lv = hp * 2 + h01
                            nc.tensor.matmul(ots[h01][:, qoff:512],
                                             vaugs[tk][:, lv * VS:lv * VS + 65],
                                             px[:, 512 * h01:512 * h01 + w],
                                             start=(tk == 0),
                                             stop=(tk == ntk - 1))
                    for h01 in range(2):
                        # normalizer: bcast sums row, approx-reciprocal, mul
                        srow = p_sr.tile([65, 512], BF16, tag="srow",
                                         name=f"sr{hp}_{qb}_{h01}")
                        nc.vector.tensor_copy(srow[64:65, :], ots[h01][64:65, :])
                        bcp = pp_mm.tile([64, 512], F32, tag="mm",
                                         name=f"bc{hp}_{qb}_{h01}")
                        nc.tensor.matmul(bcp[:], ones64[64:65, 0:64],
                                         srow[64:65, :], start=True, stop=True)
                        rcp = p_rc.tile([64, 512], F32, tag="rcp",
                                        name=f"rc{hp}_{qb}_{h01}")
                        nc.vector.reciprocal_approx_fast(rcp[:], bcp[:])
                        nc.vector.tensor_mul(yt[64 * h01:64 * h01 + 64, qsl],
                                             ots[h01][0:64, :], rcp[:])

            def emit_proj_a(yts, wps, obhs):
                # out_A = yt0.T wp0 + yt1.T wp1, kept in SBUF; emitted early
                # so the scheduler can fill hp2/hp3 attention gaps
                for t in range(TT):
                    for cc in range(2):
                        csl = slice(cc * 512, (cc + 1) * 512)
                        ps = pp_mm.tile([128, 512], F32, tag="mm")
                        for k in range(2):
                            nc.tensor.matmul(ps[:], yts[k][:, t * 128:(t + 1) * 128],
                                             wps[k][:, csl], start=(k == 0),
                                             stop=(k == 1))
                        oh = p_oh.tile([128, 512], BF16, tag="obh",
                                       name=f"oh{t}_{cc}")
                        nc.vector.tensor_copy(oh[:], ps[:])
                        obhs.append(oh)

            def emit_proj_b(yts, wps, obhs):
                for t in range(TT):
                    for cc in range(2):
                        csl = slice(cc * 512, (cc + 1) * 512)
                        ps = pp_mm.tile([128, 512], F32, tag="mm")
                        for k in range(2, PAIRS):
                            nc.tensor.matmul(ps[:], yts[k][:, t * 128:(t + 1) * 128],
                                             wps[k][:, csl], start=(k == 2),
                                             stop=(k == PAIRS - 1))
                        ob = p_ob.tile([128, 512], BF16, tag="ob")
                        nc.vector.tensor_add(ob[:], ps[:], obhs[2 * t + cc][:])
                        nc.sync.dma_start(out=out_d.ap()[t * 128:(t + 1) * 128, csl],
                                          in_=ob[:])

            for rep in range(nrep):
                # ---- load xT in column chunks so V can start early ----
                xts = []
                for k in range(KT):
                    t_ = p_xt.tile([128, T], BF16, tag="xt", name=f"xt{rep}_{k}")
                    xts.append(t_)
                for ch in range(4):
                    csl = slice(ch * 512, (ch + 1) * 512)
                    for k in range(KT):
                        nc.sync.dma_start(
                            out=xts[k][:, csl],
                            in_=xT_d.ap()[k * 128:(k + 1) * 128, csl])

                # ---- V phase: v_aug[t] [128, 8*66], natural [T, vdims] ----
                wvs = []
                for k in range(KT):
                    w = p_w.tile([128, 512], BF16, tag="w")
                    nc.sync.dma_start(
                        out=w[:], in_=wqkv_d.ap()[k * 128:(k + 1) * 128, 1024:1536])
                    wvs.append(w)
                if bias:
                    bv = p_tiny.tile([1, 512], BF16, tag="bv")
                    nc.sync.dma_start(out=bv[:],
                                      in_=bqkv_d.ap()[1024:1536].unsqueeze(0))
                vaugs = []
                for t in range(TT):
                    ps = pp_mm.tile([128, 512], F32, tag="mm")
                    for k in range(KT):
                        nc.tensor.matmul(ps[:], xts[k][:, t * 128:(t + 1) * 128],
                                         wvs[k][:], start=(k == 0),
                                         stop=(not bias and k == KT - 1))
                    if bias:
                        nc.tensor.matmul(ps[:], ones_r[:], bv[:], start=False,
                                         stop=True)
                    va = p_va.tile([128, 8 * VS], BF16, tag="vaug")
                    nc.vector.tensor_copy(
                        va[:].rearrange("p (l c) -> p l c", c=VS)[:, :, 0:64],
                        ps[:].rearrange("p (l c) -> p l c", c=64))
                    nc.sync.dma_start(
                        out=va[:].rearrange("p (l c) -> p l c", c=VS)[:, :, 64:65],
                        in_=ones_v_d.ap().unsqueeze(2))
                    vaugs.append(va)

                # ---- per head pair: QK projection then causal attention ----
                yts = []
                obhs = []
                for hp in range(PAIRS):
                    wqks = []
                    for k in range(KT):
                        wqk = p_w.tile([128, 256], BF16, tag="w", name=f"wqk{hp}_{k}")
                        nc.sync.dma_start(
                            out=wqk[:, 0:128],
                            in_=wqkv_d.ap()[k * 128:(k + 1) * 128,
                                            hp * 128:(hp + 1) * 128])
                        nc.sync.dma_start(
                            out=wqk[:, 128:256],
                            in_=wqkv_d.ap()[k * 128:(k + 1) * 128,
                                            512 + hp * 128:512 + (hp + 1) * 128])
                        wqks.append(wqk)
                    if bias:
                        bq = p_tiny.tile([128, 1], BF16, tag="bq")
                        nc.sync.dma_start(out=bq[:], in_=bqkv_d.ap()
                                          [hp * 128:(hp + 1) * 128].unsqueeze(1))
                        bk = p_tiny.tile([128, 1], BF16, tag="bk")
                        nc.sync.dma_start(out=bk[:], in_=bqkv_d.ap()
                                          [512 + hp * 128:512 + (hp + 1) * 128]
                                          .unsqueeze(1))

                    qt = p_qk.tile([128, T], BF16, tag="qt")
                    kt = p_qk.tile([128, T], BF16, tag="kt")
                    for n in range(NQ):
                        sl = slice(n * 512, (n + 1) * 512)
                        psq = pp_mm.tile([128, 512], F32, tag="mm")
                        for k in range(KT):
                            nc.tensor.matmul(psq[:], wqks[k][:, 0:128], xts[k][:, sl],
                                             start=(k == 0), stop=(k == KT - 1))
                        if bias:
                            nc.vector.tensor_scalar_add(qt[:, sl], psq[:], bq[:, 0:1])
                        else:
                            nc.vector.tensor_copy(qt[:, sl], psq[:])
                        psk = pp_mm.tile([128, 512], F32, tag="mm")
                        for k in range(KT):
                            nc.tensor.matmul(psk[:], wqks[k][:, 128:256], xts[k][:, sl],
                                             start=(k == 0), stop=(k == KT - 1))
                        if bias:
                            nc.vector.tensor_scalar_add(kt[:, sl], psk[:], bk[:, 0:1])
                        else:
                            nc.vector.tensor_copy(kt[:, sl], psk[:])

                    # attention for the two heads of this pair
                    yt = p_yt.tile([128, T], BF16, tag="yt")
                    emit_attention(hp, qt, kt, yt, vaugs, tri, ones64)
                    yts.append(yt)

                    if hp == 1:
                        wps = []
                        for k in range(PAIRS):
                            w = p_wp.tile([128, C], BF16, tag="wp")
                            nc.sync.dma_start(out=w[:],
                                              in_=wp_d.ap()[k * 128:(k + 1) * 128, :])
                            wps.append(w)
                        emit_proj_a(yts, wps, obhs)

                # ---- proj half B: add yt2/yt3 contributions and store ----
                emit_proj_b(yts, wps, obhs)
    nc.compile()
    return nc


def _get_nc(bias=False):
    key = ("nc", bias)
    if key not in _NC_CACHE:
        _NC_CACHE[key] = _build(bias=bias)
    return _NC_CACHE[key]


def kernel(x, w_attn, b_attn, w_proj, b_proj):
    x = np.asarray(x, dtype=np.float32)
    w_attn = np.asarray(w_attn, dtype=np.float32)
    b_attn = np.asarray(b_attn, dtype=np.float32)
    w_proj = np.asarray(w_proj, dtype=np.float32)
    b_proj = np.asarray(b_proj, dtype=np.float32)
    nc = _get_nc(bias=bool(np.any(b_attn)))
    bf = ml_dtypes.bfloat16
    in_maps = []
    for c in range(8):
        b, g = divmod(c, 2)
        xT = np.ascontiguousarray(x[b].T.astype(bf))
        s = 512 * g
        wqkv = np.ascontiguousarray(np.concatenate(
            [w_attn[:, s:s + 512],
             w_attn[:, 1024 + s:1024 + s + 512],
             w_attn[:, 2048 + s:2048 + s + 512]], axis=1).astype(bf))
        bqkv = np.ascontiguousarray(np.concatenate(
            [b_attn[s:s + 512], b_attn[1024 + s:1024 + s + 512],
             b_attn[2048 + s:2048 + s + 512]]).astype(bf))
        wp = np.ascontiguousarray(w_proj[s:s + 512, :].astype(bf))
        in_maps.append({"xT": xT, "wqkv": wqkv, "bqkv": bqkv, "wp": wp})
    globals()["_last_in_maps"] = in_maps
    res = run_bass_kernel_spmd(nc, in_maps, list(range(8)))
    out = np.empty((B, T, C), dtype=np.float32)
    for b in range(B):
        out[b] = (res.results[2 * b]["out"].astype(np.float32)
                  + res.results[2 * b + 1]["out"].astype(np.float32))
    out += b_proj
    return out


# revision 20
# speedup vs baseline: 1.3290x; 1.3290x over previous
"""Causal self-attention (B=4, T=2048, C=1024, 16 heads) on 8 NeuronCores.

Sharding: core c -> batch b=c//2, head group g=c%2 (8 heads each).
Each core computes q,k,v for its 8 heads, causal flash-style attention,
and a partial output projection (row-slice of w_proj). Host sums the two
partials per batch and adds b_proj.

v4: all-bf16 pipeline (fp32 matmul is half PE rate; fp8 fails the 2e-2
error gate on logit-noise tails), reciprocal_approx_fast for the softmax
divide, diagonal-refined S/exp/PV windows (only columns right of the
diagonal are computed on diagonal tiles), split projection so the first
half (head pairs 0-1) fills PE gaps during the second half's attention,
column-chunked xT loads so the V phase starts early.

Layout strategy:
  - x is pre-transposed on host -> xT [C, T] so the C-contraction sits on
    SBUF partitions for the qkv projections.
  - q, k are produced transposed (qT/kT [head_dim, T]); the two heads of a
    pair live on partitions 0-63 / 64-127 so their S^T matmuls run
    concurrently in separate PE row groups (tile_position auto-derived).
  - v is produced in natural [T, 512] layout, stored interleaved per-head
    with a ones column (66-stride for 4B alignment): the PV matmul's row 64
    accumulates the softmax denominators for free.
  - Normalization: sums row -> K=1 ones matmul broadcast -> approx
    reciprocal -> multiply into y^T.
"""
import numpy as np
import ml_dtypes
import concourse.bass as bass
from concourse import bacc
import concourse.tile as tile
import concourse.mybir as mybir
from concourse.bass_utils import run_bass_kernel_spmd

B, T, C = 4, 2048, 1024
HD = 64            # head dim
HL = 8             # local heads per core
PAIRS = 4          # local head pairs
KT = C // 128      # 8 contraction tiles for qkv
TT = T // 128      # 16 row tiles of T
NQ = T // 512      # 4 query chunks of 512
F32 = mybir.dt.float32
BF16 = mybir.dt.bfloat16
EXP = mybir.ActivationFunctionType.Exp
VS = 66            # per-head stride in vaug (64 v + 1 ones + 1 pad)

_NC_CACHE = {}


def _build(nrep=1, bias=False):
    nc = bacc.Bacc("TRN2", target_bir_lowering=False, debug=False)
    xT_d = nc.dram_tensor("xT", [C, T], BF16, kind="ExternalInput")
    wqkv_d = nc.dram_tensor("wqkv", [C, 1536], BF16, kind="ExternalInput")
    bqkv_d = nc.dram_tensor("bqkv", [1536], BF16, kind="ExternalInput")
    wp_d = nc.dram_tensor("wp", [512, C], BF16, kind="ExternalInput")
    out_d = nc.dram_tensor("out", [T, C], BF16, kind="ExternalOutput")

    # constants baked into the NEFF
    tri_np = np.zeros((128, 128), dtype=np.float32)
    for p in range(128):
        tri_np[p, p:] = 1.0
    tri_d = nc.inline_tensor(tri_np.astype(ml_dtypes.bfloat16), name="ctri")
    ones_r_d = nc.inline_tensor(
        np.ones((1, 128), dtype=ml_dtypes.bfloat16), name="ones_r")
    ones_v_d = nc.inline_tensor(
        np.ones((128, 8), dtype=ml_dtypes.bfloat16), name="ones_v")

    with tile.TileContext(nc) as tc:
        with (
            tc.tile_pool(name="xt", bufs=8) as p_xt,        # xT tiles
            tc.tile_pool(name="w", bufs=16) as p_w,         # w_v + per-pair w_q/w_k
            tc.tile_pool(name="wp", bufs=4) as p_wp,
            tc.tile_pool(name="vaug", bufs=16) as p_va,
            tc.tile_pool(name="qk", bufs=4) as p_qk,
            tc.tile_pool(name="yt", bufs=4) as p_yt,
            tc.tile_pool(name="mask", bufs=1) as p_mask,
            tc.tile_pool(name="pexp", bufs=4) as p_px,
            tc.tile_pool(name="srow", bufs=3) as p_sr,
            tc.tile_pool(name="rcp", bufs=3) as p_rc,
            tc.tile_pool(name="ob", bufs=3) as p_ob,
            tc.tile_pool(name="tiny", bufs=2) as p_tiny,
            tc.tile_pool(name="mm", bufs=2, space="PSUM") as pp_mm,
            tc.tile_pool(name="st", bufs=2, space="PSUM") as pp_st,
            tc.tile_pool(name="ot", bufs=2, space="PSUM") as pp_ot,
        ):
            # ---- constants ----
            ones_r = p_tiny.tile([1, 128], BF16, tag="onesr")
            nc.sync.dma_start(out=ones_r[:], in_=ones_r_d.ap())
            # ones row living at partition 64 (for K-dim alignment with sums)
            ones64 = p_tiny.tile([65, 128], BF16, tag="ones64")
            nc.sync.dma_start(out=ones64[64:65, :], in_=ones_r_d.ap())
            tri = p_mask.tile([128, 128], BF16, tag="mask")
            nc.sync.dma_start(out=tri[:], in_=tri_d.ap())

            def emit_attention(hp, qt, kt, yt, vaugs, tri, ones64):
                for qb in range(NQ):
                    qsl = slice(qb * 512, (qb + 1) * 512)
                    ots = [pp_ot.tile([65, 512], F32, tag="ot",
                                      name=f"ot{hp}_{qb}_{i}")
                           for i in range(2)]
                    ntk = 4 * qb + 4
                    for tk in range(ntk):
                        ksl = slice(tk * 128, (tk + 1) * 128)
                        diag_j = tk - 4 * qb
                        # diagonal tiles: only columns >= the diagonal are
                        # live; compute the [qoff:512] window only
                        qoff = 0 if diag_j < 0 else 128 * diag_j
                        w = 512 - qoff
                        # one double-bank psum holds both heads' S tiles;
                        # the two matmuls run in separate PE row groups
                        st = pp_st.tile([128, 1024], F32, tag="st")
                        for h01 in range(2):
                            prt = slice(64 * h01, 64 * h01 + 64)
                            nc.tensor.matmul(
                                st[:, 512 * h01:512 * h01 + w],
                                kt[prt, ksl],
                                qt[prt, qb * 512 + qoff:(qb + 1) * 512],
                                start=True, stop=True)
                        px = p_px.tile([128, 1024], BF16, tag="pexp")
                        stv = st[:].rearrange("p (r f) -> p r f", r=2)
                        pxv = px[:].rearrange("p (r f) -> p r f", r=2)
                        nc.scalar.activation(pxv[:, :, 0:w], stv[:, :, 0:w],
                                             EXP, scale=0.125)
                        if diag_j >= 0:
                            # triangular boundary is the first 128 columns
                            # of the window
                            m2 = tri[:].unsqueeze(1).broadcast_to([128, 2, 128])
                            nc.vector.tensor_mul(pxv[:, :, 0:128],
                                                 pxv[:, :, 0:128], m2)
                        for h01 in range(2):
                            lv = hp * 2 + h01
                            nc.tensor.matmul(ots[h01][:, qoff:512],
                                             vaugs[tk][:, lv * VS:lv * VS + 65],
                                             px[:, 512 * h01:512 * h01 + w],
                                             start=(tk == 0),
                                             stop=(tk == ntk - 1))
                    for h01 in range(2):
                        # evacuate the psum bank first (frees it for the next
                        # query block's PV), then bcast sums row, approx
                        # reciprocal, and normalize from SBUF at 2x DVE rate
                        otc = p_sr.tile([65, 512], BF16, tag="srow",
                                        name=f"oc{hp}_{qb}_{h01}")
                        nc.vector.tensor_copy(otc[:], ots[h01][:])
                        bcp = pp_mm.tile([64, 512], F32, tag="mm",
                                         name=f"bc{hp}_{qb}_{h01}")
                        nc.tensor.matmul(bcp[:], ones64[64:65, 0:64],
                                         otc[64:65, :], start=True, stop=True)
                        rcp = p_rc.tile([64, 512], F32, tag="rcp",
                                        name=f"rc{hp}_{qb}_{h01}")
                        nc.vector.reciprocal_approx_fast(rcp[:], bcp[:])
                        nc.vector.tensor_mul(yt[64 * h01:64 * h01 + 64, qsl],
                                             otc[0:64, :], rcp[:])

            def emit_proj(yts, wps):
                for t in range(TT):
                    for cc in range(2):
                        csl = slice(cc * 512, (cc + 1) * 512)
                        ps = pp_mm.tile([128, 512], F32, tag="mm")
                        for k in range(PAIRS):
                            nc.tensor.matmul(ps[:], yts[k][:, t * 128:(t + 1) * 128],
                                             wps[k][:, csl], start=(k == 0),
                                             stop=(k == PAIRS - 1))
                        ob = p_ob.tile([128, 512], BF16, tag="ob")
                        nc.vector.tensor_copy(ob[:], ps[:])
                        nc.sync.dma_start(out=out_d.ap()[t * 128:(t + 1) * 128, csl],
                                          in_=ob[:])

            for rep in range(nrep):
                # ---- load xT in column chunks so V can start early ----
                xts = []
                for k in range(KT):
                    t_ = p_xt.tile([128, T], BF16, tag="xt", name=f"xt{rep}_{k}")
                    xts.append(t_)
                for ch in range(2):
                    csl = slice(ch * 1024, (ch + 1) * 1024)
                    for k in range(KT):
                        nc.sync.dma_start(
                            out=xts[k][:, csl],
                            in_=xT_d.ap()[k * 128:(k + 1) * 128, csl])

                # ---- V phase: v_aug[t] [128, 8*66], natural [T, vdims] ----
                wvs = []
                for k in range(KT):
                    w = p_w.tile([128, 512], BF16, tag="w")
                    nc.sync.dma_start(
                        out=w[:], in_=wqkv_d.ap()[k * 128:(k + 1) * 128, 1024:1536])
                    wvs.append(w)
                if bias:
                    bv = p_tiny.tile([1, 512], BF16, tag="bv")
                    nc.sync.dma_start(out=bv[:],
                                      in_=bqkv_d.ap()[1024:1536].unsqueeze(0))
                vaugs = []
                for t in range(TT):
                    ps = pp_mm.tile([128, 512], F32, tag="mm")
                    for k in range(KT):
                        nc.tensor.matmul(ps[:], xts[k][:, t * 128:(t + 1) * 128],
                                         wvs[k][:], start=(k == 0),
                                         stop=(not bias and k == KT - 1))
                    if bias:
                        nc.tensor.matmul(ps[:], ones_r[:], bv[:], start=False,
                                         stop=True)
                    va = p_va.tile([128, 8 * VS], BF16, tag="vaug")
                    nc.vector.tensor_copy(
                        va[:].rearrange("p (l c) -> p l c", c=VS)[:, :, 0:64],
                        ps[:].rearrange("p (l c) -> p l c", c=64))
                    nc.sync.dma_start(
                        out=va[:].rearrange("p (l c) -> p l c", c=VS)[:, :, 64:65],
                        in_=ones_v_d.ap().unsqueeze(2))
                    vaugs.append(va)

                # ---- per head pair: QK projection then causal attention ----
                yts = []
                for hp in range(PAIRS):
                    wqks = []
                    for k in range(KT):
                        wqk = p_w.tile([128, 256], BF16, tag="w", name=f"wqk{hp}_{k}")
                        nc.sync.dma_start(
                            out=wqk[:, 0:128],
                            in_=wqkv_d.ap()[k * 128:(k + 1) * 128,
                                            hp * 128:(hp + 1) * 128])
                        nc.sync.dma_start(
                            out=wqk[:, 128:256],
                            in_=wqkv_d.ap()[k * 128:(k + 1) * 128,
                                            512 + hp * 128:512 + (hp + 1) * 128])
                        wqks.append(wqk)
                    if bias:
                        bq = p_tiny.tile([128, 1], BF16, tag="bq")
                        nc.sync.dma_start(out=bq[:], in_=bqkv_d.ap()
                                          [hp * 128:(hp + 1) * 128].unsqueeze(1))
                        bk = p_tiny.tile([128, 1], BF16, tag="bk")
                        nc.sync.dma_start(out=bk[:], in_=bqkv_d.ap()
                                          [512 + hp * 128:512 + (hp + 1) * 128]
                                          .unsqueeze(1))

                    qt = p_qk.tile([128, T], BF16, tag="qt")
                    kt = p_qk.tile([128, T], BF16, tag="kt")
                    for n in range(NQ):
                        sl = slice(n * 512, (n + 1) * 512)
                        psq = pp_mm.tile([128, 512], F32, tag="mm")
                        for k in range(KT):
                            nc.tensor.matmul(psq[:], wqks[k][:, 0:128], xts[k][:, sl],
                                             start=(k == 0), stop=(k == KT - 1))
                        if bias:
                            nc.vector.tensor_scalar_add(qt[:, sl], psq[:], bq[:, 0:1])
                        else:
                            nc.vector.tensor_copy(qt[:, sl], psq[:])
                        psk = pp_mm.tile([128, 512], F32, tag="mm")
                        for k in range(KT):
                            nc.tensor.matmul(psk[:], wqks[k][:, 128:256], xts[k][:, sl],
                                             start=(k == 0), stop=(k == KT - 1))
                        if bias:
                            nc.vector.tensor_scalar_add(kt[:, sl], psk[:], bk[:, 0:1])
                        else:
                            nc.vector.tensor_copy(kt[:, sl], psk[:])

                    # attention for the two heads of this pair
                    yt = p_yt.tile([128, T], BF16, tag="yt")
                    emit_attention(hp, qt, kt, yt, vaugs, tri, ones64)
                    yts.append(yt)

                    if hp == 1:
                        wps = []
                        for k in range(PAIRS):
                            w = p_wp.tile([128, C], BF16, tag="wp")
                            nc.sync.dma_start(out=w[:],
                                              in_=wp_d.ap()[k * 128:(k + 1) * 128, :])
                            wps.append(w)

                # ---- projection: out[t, c] = sum_k yt_k[:, t].T @ wp_k ----
                emit_proj(yts, wps)
    nc.compile()
    return nc


def _get_nc(bias=False):
    key = ("nc", bias)
    if key not in _NC_CACHE:
        _NC_CACHE[key] = _build(bias=bias)
    return _NC_CACHE[key]


def kernel(x, w_attn, b_attn, w_proj, b_proj):
    x = np.asarray(x, dtype=np.float32)
    w_attn = np.asarray(w_attn, dtype=np.float32)
    b_attn = np.asarray(b_attn, dtype=np.float32)
    w_proj = np.asarray(w_proj, dtype=np.float32)
    b_proj = np.asarray(b_proj, dtype=np.float32)
    nc = _get_nc(bias=bool(np.any(b_attn)))
    bf = ml_dtypes.bfloat16
    in_maps = []
    for c in range(8):
        b, g = divmod(c, 2)
        xT = np.ascontiguousarray(x[b].T.astype(bf))
        s = 512 * g
        wqkv = np.ascontiguousarray(np.concatenate(
            [w_attn[:, s:s + 512],
             w_attn[:, 1024 + s:1024 + s + 512],
             w_attn[:, 2048 + s:2048 + s + 512]], axis=1).astype(bf))
        bqkv = np.ascontiguousarray(np.concatenate(
            [b_attn[s:s + 512], b_attn[1024 + s:1024 + s + 512],
             b_attn[2048 + s:2048 + s + 512]]).astype(bf))
        wp = np.ascontiguousarray(w_proj[s:s + 512, :].astype(bf))
        in_maps.append({"xT": xT, "wqkv": wqkv, "bqkv": bqkv, "wp": wp})
    globals()["_last_in_maps"] = in_maps
    res = run_bass_kernel_spmd(nc, in_maps, list(range(8)))
    out = np.empty((B, T, C), dtype=np.float32)
    for b in range(B):
        out[b] = (res.results[2 * b]["out"].astype(np.float32)
                  + res.results[2 * b + 1]["out"].astype(np.float32))
    out += b_proj
    return out


# revision 21
# speedup vs baseline: 1.3782x; 1.0370x over previous
"""Causal self-attention (B=4, T=2048, C=1024, 16 heads) on 8 NeuronCores.

Sharding: core c -> batch b=c//2, head group g=c%2 (8 heads each).
Each core computes q,k,v for its 8 heads, causal flash-style attention,
and a partial output projection (row-slice of w_proj). Host sums the two
partials per batch and adds b_proj.

v4: all-bf16 pipeline (fp32 matmul is half PE rate; fp8 fails the 2e-2
error gate on logit-noise tails), reciprocal_approx_fast for the softmax
divide, diagonal-refined S/exp/PV windows (only columns right of the
diagonal are computed on diagonal tiles), split projection so the first
half (head pairs 0-1) fills PE gaps during the second half's attention,
column-chunked xT loads so the V phase starts early.

Layout strategy:
  - x is pre-transposed on host -> xT [C, T] so the C-contraction sits on
    SBUF partitions for the qkv projections.
  - q, k are produced transposed (qT/kT [head_dim, T]); the two heads of a
    pair live on partitions 0-63 / 64-127 so their S^T matmuls run
    concurrently in separate PE row groups (tile_position auto-derived).
  - v is produced in natural [T, 512] layout, stored interleaved per-head
    with a ones column (66-stride for 4B alignment): the PV matmul's row 64
    accumulates the softmax denominators for free.
  - Normalization: sums row -> K=1 ones matmul broadcast -> approx
    reciprocal -> multiply into y^T.
"""
import numpy as np
import ml_dtypes
import concourse.bass as bass
from concourse import bacc
import concourse.tile as tile
import concourse.mybir as mybir
from concourse.bass_utils import run_bass_kernel_spmd

B, T, C = 4, 2048, 1024
HD = 64            # head dim
HL = 8             # local heads per core
PAIRS = 4          # local head pairs
KT = C // 128      # 8 contraction tiles for qkv
TT = T // 128      # 16 row tiles of T
NQ = T // 512      # 4 query chunks of 512
F32 = mybir.dt.float32
BF16 = mybir.dt.bfloat16
EXP = mybir.ActivationFunctionType.Exp
VS = 66            # per-head stride in vaug (64 v + 1 ones + 1 pad)

_NC_CACHE = {}


def _build(nrep=1, bias=False):
    nc = bacc.Bacc("TRN2", target_bir_lowering=False, debug=False)
    xT_d = nc.dram_tensor("xT", [C, T], BF16, kind="ExternalInput")
    wqkv_d = nc.dram_tensor("wqkv", [C, 1536], BF16, kind="ExternalInput")
    bqkv_d = nc.dram_tensor("bqkv", [1536], BF16, kind="ExternalInput")
    wp_d = nc.dram_tensor("wp", [512, C], BF16, kind="ExternalInput")
    out_d = nc.dram_tensor("out", [T, C], BF16, kind="ExternalOutput")

    # constants baked into the NEFF
    tri_np = np.zeros((128, 128), dtype=np.float32)
    for p in range(128):
        tri_np[p, p:] = 1.0
    tri_d = nc.inline_tensor(tri_np.astype(ml_dtypes.bfloat16), name="ctri")
    ones_r_d = nc.inline_tensor(
        np.ones((1, 128), dtype=ml_dtypes.bfloat16), name="ones_r")
    ones_v_d = nc.inline_tensor(
        np.ones((128, 8), dtype=ml_dtypes.bfloat16), name="ones_v")

    with tile.TileContext(nc) as tc:
        with (
            tc.tile_pool(name="xt", bufs=8) as p_xt,        # xT tiles
            tc.tile_pool(name="w", bufs=16) as p_w,         # w_v + per-pair w_q/w_k
            tc.tile_pool(name="wp", bufs=4) as p_wp,
            tc.tile_pool(name="vaug", bufs=16) as p_va,
            tc.tile_pool(name="qk", bufs=4) as p_qk,
            tc.tile_pool(name="yt", bufs=4) as p_yt,
            tc.tile_pool(name="mask", bufs=1) as p_mask,
            tc.tile_pool(name="pexp", bufs=4) as p_px,
            tc.tile_pool(name="srow", bufs=3) as p_sr,
            tc.tile_pool(name="rcp", bufs=3) as p_rc,
            tc.tile_pool(name="ob", bufs=3) as p_ob,
            tc.tile_pool(name="tiny", bufs=2) as p_tiny,
            tc.tile_pool(name="mm", bufs=2, space="PSUM") as pp_mm,
            tc.tile_pool(name="st", bufs=2, space="PSUM") as pp_st,
            tc.tile_pool(name="ot", bufs=2, space="PSUM") as pp_ot,
        ):
            # ---- constants ----
            ones_r = p_tiny.tile([1, 128], BF16, tag="onesr")
            nc.sync.dma_start(out=ones_r[:], in_=ones_r_d.ap())
            # ones row living at partition 64 (for K-dim alignment with sums)
            ones64 = p_tiny.tile([65, 128], BF16, tag="ones64")
            nc.sync.dma_start(out=ones64[64:65, :], in_=ones_r_d.ap())
            tri = p_mask.tile([128, 128], BF16, tag="mask")
            nc.sync.dma_start(out=tri[:], in_=tri_d.ap())

            def emit_attention(hp, qt, kt, yt, vaugs, tri, ones64):
                for qb in range(NQ):
                    qsl = slice(qb * 512, (qb + 1) * 512)
                    ots = [pp_ot.tile([65, 512], F32, tag="ot",
                                      name=f"ot{hp}_{qb}_{i}")
                           for i in range(2)]
                    ntk = 4 * qb + 4
                    for tk in range(ntk):
                        ksl = slice(tk * 128, (tk + 1) * 128)
                        diag_j = tk - 4 * qb
                        # diagonal tiles: only columns >= the diagonal are
                        # live; compute the [qoff:512] window only
                        qoff = 0 if diag_j < 0 else 128 * diag_j
                        w = 512 - qoff
                        # one double-bank psum holds both heads' S tiles;
                        # the two matmuls run in separate PE row groups
                        st = pp_st.tile([128, 1024], F32, tag="st")
                        for h01 in range(2):
                            prt = slice(64 * h01, 64 * h01 + 64)
                            nc.tensor.matmul(
                                st[:, 512 * h01:512 * h01 + w],
                                kt[prt, ksl],
                                qt[prt, qb * 512 + qoff:(qb + 1) * 512],
                                start=True, stop=True)
                        px = p_px.tile([128, 1024], BF16, tag="pexp")
                        stv = st[:].rearrange("p (r f) -> p r f", r=2)
                        pxv = px[:].rearrange("p (r f) -> p r f", r=2)
                        nc.scalar.activation(pxv[:, :, 0:w], stv[:, :, 0:w],
                                             EXP, scale=0.125)
                        if diag_j >= 0:
                            # triangular boundary is the first 128 columns
                            # of the window
                            m2 = tri[:].unsqueeze(1).broadcast_to([128, 2, 128])
                            nc.vector.tensor_mul(pxv[:, :, 0:128],
                                                 pxv[:, :, 0:128], m2)
                        for h01 in range(2):
                            lv = hp * 2 + h01
                            nc.tensor.matmul(ots[h01][:, qoff:512],
                                             vaugs[tk][:, lv * VS:lv * VS + 65],
                                             px[:, 512 * h01:512 * h01 + w],
                                             start=(tk == 0),
                                             stop=(tk == ntk - 1))
                    for h01 in range(2):
                        # evacuate the psum bank first (frees it for the next
                        # query block's PV), then bcast sums row, approx
                        # reciprocal, and normalize from SBUF at 2x DVE rate
                        otc = p_sr.tile([65, 512], BF16, tag="srow",
                                        name=f"oc{hp}_{qb}_{h01}")
                        nc.vector.tensor_copy(otc[:], ots[h01][:])
                        bcp = pp_mm.tile([64, 512], F32, tag="mm",
                                         name=f"bc{hp}_{qb}_{h01}")
                        nc.tensor.matmul(bcp[:], ones64[64:65, 0:64],
                                         otc[64:65, :], start=True, stop=True)
                        rcp = p_rc.tile([64, 512], F32, tag="rcp",
                                        name=f"rc{hp}_{qb}_{h01}")
                        nc.vector.reciprocal_approx_fast(rcp[:], bcp[:])
                        nc.vector.tensor_mul(yt[64 * h01:64 * h01 + 64, qsl],
                                             otc[0:64, :], rcp[:])

            def emit_proj(yts, wps):
                for t in range(TT):
                    for cc in range(2):
                        csl = slice(cc * 512, (cc + 1) * 512)
                        ps = pp_mm.tile([128, 512], F32, tag="mm")
                        for k in range(PAIRS):
                            nc.tensor.matmul(ps[:], yts[k][:, t * 128:(t + 1) * 128],
                                             wps[k][:, csl], start=(k == 0),
                                             stop=(k == PAIRS - 1))
                        ob = p_ob.tile([128, 512], BF16, tag="ob")
                        nc.vector.tensor_copy(ob[:], ps[:])
                        nc.sync.dma_start(out=out_d.ap()[t * 128:(t + 1) * 128, csl],
                                          in_=ob[:])

            def emit_v(vaugs, xts, wvs, bv, lo, hi):
                for t in range(lo, hi):
                    ps = pp_mm.tile([128, 512], F32, tag="mm")
                    for k in range(KT):
                        nc.tensor.matmul(ps[:], xts[k][:, t * 128:(t + 1) * 128],
                                         wvs[k][:], start=(k == 0),
                                         stop=(bv is None and k == KT - 1))
                    if bv is not None:
                        nc.tensor.matmul(ps[:], ones_r[:], bv[:], start=False,
                                         stop=True)
                    va = p_va.tile([128, 8 * VS], BF16, tag="vaug")
                    nc.vector.tensor_copy(
                        va[:].rearrange("p (l c) -> p l c", c=VS)[:, :, 0:64],
                        ps[:].rearrange("p (l c) -> p l c", c=64))
                    nc.sync.dma_start(
                        out=va[:].rearrange("p (l c) -> p l c", c=VS)[:, :, 64:65],
                        in_=ones_v_d.ap().unsqueeze(2))
                    vaugs.append(va)

            for rep in range(nrep):
                # ---- wv first, then xT column-halves interleaved with the V
                # matmuls that consume them, so compute starts ~6us earlier
                wvs = []
                for k in range(KT):
                    w = p_w.tile([128, 512], BF16, tag="w")
                    nc.sync.dma_start(
                        out=w[:], in_=wqkv_d.ap()[k * 128:(k + 1) * 128, 1024:1536])
                    wvs.append(w)
                bv = None
                if bias:
                    bv = p_tiny.tile([1, 512], BF16, tag="bv")
                    nc.sync.dma_start(out=bv[:],
                                      in_=bqkv_d.ap()[1024:1536].unsqueeze(0))
                xts = []
                for k in range(KT):
                    t_ = p_xt.tile([128, T], BF16, tag="xt", name=f"xt{rep}_{k}")
                    xts.append(t_)
                for k in range(KT):
                    nc.sync.dma_start(out=xts[k][:, 0:1024],
                                      in_=xT_d.ap()[k * 128:(k + 1) * 128, 0:1024])
                vaugs = []
                emit_v(vaugs, xts, wvs, bv, 0, TT // 2)
                for k in range(KT):
                    nc.sync.dma_start(out=xts[k][:, 1024:2048],
                                      in_=xT_d.ap()[k * 128:(k + 1) * 128, 1024:2048])
                emit_v(vaugs, xts, wvs, bv, TT // 2, TT)

                # ---- per head pair: QK projection then causal attention ----
                yts = []
                for hp in range(PAIRS):
                    wqks = []
                    for k in range(KT):
                        wqk = p_w.tile([128, 256], BF16, tag="w", name=f"wqk{hp}_{k}")
                        nc.sync.dma_start(
                            out=wqk[:, 0:128],
                            in_=wqkv_d.ap()[k * 128:(k + 1) * 128,
                                            hp * 128:(hp + 1) * 128])
                        nc.sync.dma_start(
                            out=wqk[:, 128:256],
                            in_=wqkv_d.ap()[k * 128:(k + 1) * 128,
                                            512 + hp * 128:512 + (hp + 1) * 128])
                        wqks.append(wqk)
                    if bias:
                        bq = p_tiny.tile([128, 1], BF16, tag="bq")
                        nc.sync.dma_start(out=bq[:], in_=bqkv_d.ap()
                                          [hp * 128:(hp + 1) * 128].unsqueeze(1))
                        bk = p_tiny.tile([128, 1], BF16, tag="bk")
                        nc.sync.dma_start(out=bk[:], in_=bqkv_d.ap()
                                          [512 + hp * 128:512 + (hp + 1) * 128]
                                          .unsqueeze(1))

                    qt = p_qk.tile([128, T], BF16, tag="qt")
                    kt = p_qk.tile([128, T], BF16, tag="kt")
                    for n in range(NQ):
                        sl = slice(n * 512, (n + 1) * 512)
                        psq = pp_mm.tile([128, 512], F32, tag="mm")
                        for k in range(KT):
                            nc.tensor.matmul(psq[:], wqks[k][:, 0:128], xts[k][:, sl],
                                             start=(k == 0), stop=(k == KT - 1))
                        if bias:
                            nc.vector.tensor_scalar_add(qt[:, sl], psq[:], bq[:, 0:1])
                        else:
                            nc.vector.tensor_copy(qt[:, sl], psq[:])
                        psk = pp_mm.tile([128, 512], F32, tag="mm")
                        for k in range(KT):
                            nc.tensor.matmul(psk[:], wqks[k][:, 128:256], xts[k][:, sl],
                                             start=(k == 0), stop=(k == KT - 1))
                        if bias:
                            nc.vector.tensor_scalar_add(kt[:, sl], psk[:], bk[:, 0:1])
                        else:
                            nc.vector.tensor_copy(kt[:, sl], psk[:])

                    # attention for the two heads of this pair
                    yt = p_yt.tile([128, T], BF16, tag="yt")
                    emit_attention(hp, qt, kt, yt, vaugs, tri, ones64)
                    yts.append(yt)

                    if hp == 1:
                        wps = []
                        for k in range(PAIRS):
                            w = p_wp.tile([128, C], BF16, tag="wp")
                            nc.sync.dma_start(out=w[:],
                                              in_=wp_d.ap()[k * 128:(k + 1) * 128, :])
                            wps.append(w)

                # ---- projection: out[t, c] = sum_k yt_k[:, t].T @ wp_k ----
                emit_proj(yts, wps)
    nc.compile()
    return nc


def _get_nc(bias=False):
    key = ("nc", bias)
    if key not in _NC_CACHE:
        _NC_CACHE[key] = _build(bias=bias)
    return _NC_CACHE[key]


def kernel(x, w_attn, b_attn, w_proj, b_proj):
    x = np.asarray(x, dtype=np.float32)
    w_attn = np.asarray(w_attn, dtype=np.float32)
    b_attn = np.asarray(b_attn, dtype=np.float32)
    w_proj = np.asarray(w_proj, dtype=np.float32)
    b_proj = np.asarray(b_proj, dtype=np.float32)
    nc = _get_nc(bias=bool(np.any(b_attn)))
    bf = ml_dtypes.bfloat16
    in_maps = []
    for c in range(8):
        b, g = divmod(c, 2)
        xT = np.ascontiguousarray(x[b].T.astype(bf))
        s = 512 * g
        wqkv = np.ascontiguousarray(np.concatenate(
            [w_attn[:, s:s + 512],
             w_attn[:, 1024 + s:1024 + s + 512],
             w_attn[:, 2048 + s:2048 + s + 512]], axis=1).astype(bf))
        bqkv = np.ascontiguousarray(np.concatenate(
            [b_attn[s:s + 512], b_attn[1024 + s:1024 + s + 512],
             b_attn[2048 + s:2048 + s + 512]]).astype(bf))
        wp = np.ascontiguousarray(w_proj[s:s + 512, :].astype(bf))
        in_maps.append({"xT": xT, "wqkv": wqkv, "bqkv": bqkv, "wp": wp})
    globals()["_last_in_maps"] = in_maps
    res = run_bass_kernel_spmd(nc, in_maps, list(range(8)))
    out = np.empty((B, T, C), dtype=np.float32)
    for b in range(B):
        out[b] = (res.results[2 * b]["out"].astype(np.float32)
                  + res.results[2 * b + 1]["out"].astype(np.float32))
    out += b_proj
    return out


# revision 22
# speedup vs baseline: 1.3878x; 1.0069x over previous
"""Causal self-attention (B=4, T=2048, C=1024, 16 heads) on 8 NeuronCores.

Sharding: core c -> batch b=c//2, head group g=c%2 (8 heads each).
Each core computes q,k,v for its 8 heads, causal flash-style attention,
and a partial output projection (row-slice of w_proj). Host sums the two
partials per batch and adds b_proj.

v4: all-bf16 pipeline (fp32 matmul is half PE rate; fp8 fails the 2e-2
error gate on logit-noise tails), reciprocal_approx_fast for the softmax
divide, diagonal-refined S/exp/PV windows (only columns right of the
diagonal are computed on diagonal tiles), split projection so the first
half (head pairs 0-1) fills PE gaps during the second half's attention,
column-chunked xT loads so the V phase starts early.

Layout strategy:
  - x is pre-transposed on host -> xT [C, T] so the C-contraction sits on
    SBUF partitions for the qkv projections.
  - q, k are produced transposed (qT/kT [head_dim, T]); the two heads of a
    pair live on partitions 0-63 / 64-127 so their S^T matmuls run
    concurrently in separate PE row groups (tile_position auto-derived).
  - v is produced in natural [T, 512] layout, stored interleaved per-head
    with a ones column (66-stride for 4B alignment): the PV matmul's row 64
    accumulates the softmax denominators for free.
  - Normalization: sums row -> K=1 ones matmul broadcast -> approx
    reciprocal -> multiply into y^T.
"""
import numpy as np
import ml_dtypes
import concourse.bass as bass
from concourse import bacc
import concourse.tile as tile
import concourse.mybir as mybir
from concourse.bass_utils import run_bass_kernel_spmd

B, T, C = 4, 2048, 1024
HD = 64            # head dim
HL = 8             # local heads per core
PAIRS = 4          # local head pairs
KT = C // 128      # 8 contraction tiles for qkv
TT = T // 128      # 16 row tiles of T
NQ = T // 512      # 4 query chunks of 512
F32 = mybir.dt.float32
BF16 = mybir.dt.bfloat16
EXP = mybir.ActivationFunctionType.Exp
VS = 66            # per-head stride in vaug (64 v + 1 ones + 1 pad)

_NC_CACHE = {}


def _build(nrep=1, bias=False):
    nc = bacc.Bacc("TRN2", target_bir_lowering=False, debug=False)
    xT_d = nc.dram_tensor("xT", [C, T], BF16, kind="ExternalInput")
    wqkv_d = nc.dram_tensor("wqkv", [C, 1536], BF16, kind="ExternalInput")
    bqkv_d = nc.dram_tensor("bqkv", [1536], BF16, kind="ExternalInput")
    wp_d = nc.dram_tensor("wp", [512, C], BF16, kind="ExternalInput")
    out_d = nc.dram_tensor("out", [T, C], BF16, kind="ExternalOutput")

    # constants baked into the NEFF
    tri_np = np.zeros((128, 128), dtype=np.float32)
    for p in range(128):
        tri_np[p, p:] = 1.0
    tri_d = nc.inline_tensor(tri_np.astype(ml_dtypes.bfloat16), name="ctri")
    ones_r_d = nc.inline_tensor(
        np.ones((1, 128), dtype=ml_dtypes.bfloat16), name="ones_r")
    ones_v_d = nc.inline_tensor(
        np.ones((128, 8), dtype=ml_dtypes.bfloat16), name="ones_v")

    with tile.TileContext(nc) as tc:
        with (
            tc.tile_pool(name="xt", bufs=8) as p_xt,        # xT tiles
            tc.tile_pool(name="w", bufs=16) as p_w,         # w_v + per-pair w_q/w_k
            tc.tile_pool(name="wp", bufs=4) as p_wp,
            tc.tile_pool(name="vaug", bufs=16) as p_va,
            tc.tile_pool(name="qk", bufs=4) as p_qk,
            tc.tile_pool(name="yt", bufs=4) as p_yt,
            tc.tile_pool(name="mask", bufs=1) as p_mask,
            tc.tile_pool(name="pexp", bufs=4) as p_px,
            tc.tile_pool(name="srow", bufs=3) as p_sr,
            tc.tile_pool(name="rcp", bufs=3) as p_rc,
            tc.tile_pool(name="ob", bufs=3) as p_ob,
            tc.tile_pool(name="tiny", bufs=2) as p_tiny,
            tc.tile_pool(name="mm", bufs=2, space="PSUM") as pp_mm,
            tc.tile_pool(name="st", bufs=2, space="PSUM") as pp_st,
            tc.tile_pool(name="ot", bufs=2, space="PSUM") as pp_ot,
        ):
            # ---- constants ----
            ones_r = p_tiny.tile([1, 128], BF16, tag="onesr")
            nc.sync.dma_start(out=ones_r[:], in_=ones_r_d.ap())
            # ones row living at partition 64 (for K-dim alignment with sums)
            ones64 = p_tiny.tile([65, 128], BF16, tag="ones64")
            nc.sync.dma_start(out=ones64[64:65, :], in_=ones_r_d.ap())
            tri = p_mask.tile([128, 128], BF16, tag="mask")
            nc.sync.dma_start(out=tri[:], in_=tri_d.ap())

            def emit_attention(hp, qt, kt, yt, vaugs, tri, ones64):
                for qb in range(NQ):
                    qsl = slice(qb * 512, (qb + 1) * 512)
                    ots = [pp_ot.tile([65, 512], F32, tag="ot",
                                      name=f"ot{hp}_{qb}_{i}")
                           for i in range(2)]
                    ntk = 4 * qb + 4

                    def emit_s_exp(tk):
                        ksl = slice(tk * 128, (tk + 1) * 128)
                        diag_j = tk - 4 * qb
                        # diagonal tiles: only columns >= the diagonal are
                        # live; compute the [qoff:512] window only
                        qoff = 0 if diag_j < 0 else 128 * diag_j
                        w = 512 - qoff
                        # one double-bank psum holds both heads' S tiles;
                        # the two matmuls run in separate PE row groups
                        st = pp_st.tile([128, 1024], F32, tag="st")
                        for h01 in range(2):
                            prt = slice(64 * h01, 64 * h01 + 64)
                            nc.tensor.matmul(
                                st[:, 512 * h01:512 * h01 + w],
                                kt[prt, ksl],
                                qt[prt, qb * 512 + qoff:(qb + 1) * 512],
                                start=True, stop=True)
                        px = p_px.tile([128, 1024], BF16, tag="pexp")
                        stv = st[:].rearrange("p (r f) -> p r f", r=2)
                        pxv = px[:].rearrange("p (r f) -> p r f", r=2)
                        nc.scalar.activation(pxv[:, :, 0:w], stv[:, :, 0:w],
                                             EXP, scale=0.125)
                        if diag_j >= 0:
                            # triangular boundary is the first 128 columns
                            # of the window
                            m2 = tri[:].unsqueeze(1).broadcast_to([128, 2, 128])
                            nc.vector.tensor_mul(pxv[:, :, 0:128],
                                                 pxv[:, :, 0:128], m2)
                        return tk, qoff, w, px

                    def emit_pv(info):
                        tk, qoff, w, px = info
                        for h01 in range(2):
                            lv = hp * 2 + h01
                            nc.tensor.matmul(ots[h01][:, qoff:512],
                                             vaugs[tk][:, lv * VS:lv * VS + 65],
                                             px[:, 512 * h01:512 * h01 + w],
                                             start=(tk == 0),
                                             stop=(tk == ntk - 1))

                    # software-pipelined: S/exp of tile tk+1 is emitted before
                    # PV of tile tk, so the PE fills the exp wait with S work
                    prev = emit_s_exp(0)
                    for tk in range(1, ntk):
                        cur = emit_s_exp(tk)
                        emit_pv(prev)
                        prev = cur
                    emit_pv(prev)
                    for h01 in range(2):
                        # evacuate the psum bank first (frees it for the next
                        # query block's PV), then bcast sums row, approx
                        # reciprocal, and normalize from SBUF at 2x DVE rate
                        otc = p_sr.tile([65, 512], BF16, tag="srow",
                                        name=f"oc{hp}_{qb}_{h01}")
                        nc.vector.tensor_copy(otc[:], ots[h01][:])
                        bcp = pp_mm.tile([64, 512], F32, tag="mm",
                                         name=f"bc{hp}_{qb}_{h01}")
                        nc.tensor.matmul(bcp[:], ones64[64:65, 0:64],
                                         otc[64:65, :], start=True, stop=True)
                        rcp = p_rc.tile([64, 512], F32, tag="rcp",
                                        name=f"rc{hp}_{qb}_{h01}")
                        nc.vector.reciprocal_approx_fast(rcp[:], bcp[:])
                        nc.vector.tensor_mul(yt[64 * h01:64 * h01 + 64, qsl],
                                             otc[0:64, :], rcp[:])

            def emit_proj(yts, wps):
                for t in range(TT):
                    for cc in range(2):
                        csl = slice(cc * 512, (cc + 1) * 512)
                        ps = pp_mm.tile([128, 512], F32, tag="mm")
                        for k in range(PAIRS):
                            nc.tensor.matmul(ps[:], yts[k][:, t * 128:(t + 1) * 128],
                                             wps[k][:, csl], start=(k == 0),
                                             stop=(k == PAIRS - 1))
                        ob = p_ob.tile([128, 512], BF16, tag="ob")
                        nc.vector.tensor_copy(ob[:], ps[:])
                        nc.sync.dma_start(out=out_d.ap()[t * 128:(t + 1) * 128, csl],
                                          in_=ob[:])

            def emit_v(vaugs, xts, wvs, bv, lo, hi):
                for t in range(lo, hi):
                    ps = pp_mm.tile([128, 512], F32, tag="mm")
                    for k in range(KT):
                        nc.tensor.matmul(ps[:], xts[k][:, t * 128:(t + 1) * 128],
                                         wvs[k][:], start=(k == 0),
                                         stop=(bv is None and k == KT - 1))
                    if bv is not None:
                        nc.tensor.matmul(ps[:], ones_r[:], bv[:], start=False,
                                         stop=True)
                    va = p_va.tile([128, 8 * VS], BF16, tag="vaug")
                    nc.vector.tensor_copy(
                        va[:].rearrange("p (l c) -> p l c", c=VS)[:, :, 0:64],
                        ps[:].rearrange("p (l c) -> p l c", c=64))
                    nc.sync.dma_start(
                        out=va[:].rearrange("p (l c) -> p l c", c=VS)[:, :, 64:65],
                        in_=ones_v_d.ap().unsqueeze(2))
                    vaugs.append(va)

            for rep in range(nrep):
                # ---- wv first, then xT column-halves interleaved with the V
                # matmuls that consume them, so compute starts ~6us earlier
                wvs = []
                for k in range(KT):
                    w = p_w.tile([128, 512], BF16, tag="w")
                    nc.sync.dma_start(
                        out=w[:], in_=wqkv_d.ap()[k * 128:(k + 1) * 128, 1024:1536])
                    wvs.append(w)
                bv = None
                if bias:
                    bv = p_tiny.tile([1, 512], BF16, tag="bv")
                    nc.sync.dma_start(out=bv[:],
                                      in_=bqkv_d.ap()[1024:1536].unsqueeze(0))
                xts = []
                for k in range(KT):
                    t_ = p_xt.tile([128, T], BF16, tag="xt", name=f"xt{rep}_{k}")
                    xts.append(t_)
                for k in range(KT):
                    nc.sync.dma_start(out=xts[k][:, 0:1024],
                                      in_=xT_d.ap()[k * 128:(k + 1) * 128, 0:1024])
                vaugs = []
                emit_v(vaugs, xts, wvs, bv, 0, TT // 2)
                for k in range(KT):
                    nc.sync.dma_start(out=xts[k][:, 1024:2048],
                                      in_=xT_d.ap()[k * 128:(k + 1) * 128, 1024:2048])
                emit_v(vaugs, xts, wvs, bv, TT // 2, TT)

                # ---- per head pair: QK projection then causal attention ----
                yts = []
                for hp in range(PAIRS):
                    wqks = []
                    for k in range(KT):
                        wqk = p_w.tile([128, 256], BF16, tag="w", name=f"wqk{hp}_{k}")
                        nc.sync.dma_start(
                            out=wqk[:, 0:128],
                            in_=wqkv_d.ap()[k * 128:(k + 1) * 128,
                                            hp * 128:(hp + 1) * 128])
                        nc.sync.dma_start(
                            out=wqk[:, 128:256],
                            in_=wqkv_d.ap()[k * 128:(k + 1) * 128,
                                            512 + hp * 128:512 + (hp + 1) * 128])
                        wqks.append(wqk)
                    if bias:
                        bq = p_tiny.tile([128, 1], BF16, tag="bq")
                        nc.sync.dma_start(out=bq[:], in_=bqkv_d.ap()
                                          [hp * 128:(hp + 1) * 128].unsqueeze(1))
                        bk = p_tiny.tile([128, 1], BF16, tag="bk")
                        nc.sync.dma_start(out=bk[:], in_=bqkv_d.ap()
                                          [512 + hp * 128:512 + (hp + 1) * 128]
                                          .unsqueeze(1))

                    qt = p_qk.tile([128, T], BF16, tag="qt")
                    kt = p_qk.tile([128, T], BF16, tag="kt")
                    for n in range(NQ):
                        sl = slice(n * 512, (n + 1) * 512)
                        psq = pp_mm.tile([128, 512], F32, tag="mm")
                        for k in range(KT):
                            nc.tensor.matmul(psq[:], wqks[k][:, 0:128], xts[k][:, sl],
                                             start=(k == 0), stop=(k == KT - 1))
                        if bias:
                            nc.vector.tensor_scalar_add(qt[:, sl], psq[:], bq[:, 0:1])
                        else:
                            nc.vector.tensor_copy(qt[:, sl], psq[:])
                        psk = pp_mm.tile([128, 512], F32, tag="mm")
                        for k in range(KT):
                            nc.tensor.matmul(psk[:], wqks[k][:, 128:256], xts[k][:, sl],
                                             start=(k == 0), stop=(k == KT - 1))
                        if bias:
                            nc.vector.tensor_scalar_add(kt[:, sl], psk[:], bk[:, 0:1])
                        else:
                            nc.vector.tensor_copy(kt[:, sl], psk[:])

                    # attention for the two heads of this pair
                    yt = p_yt.tile([128, T], BF16, tag="yt")
                    emit_attention(hp, qt, kt, yt, vaugs, tri, ones64)
                    yts.append(yt)

                    if hp == 1:
                        wps = []
                        for k in range(PAIRS):
                            w = p_wp.tile([128, C], BF16, tag="wp")
                            nc.sync.dma_start(out=w[:],
                                              in_=wp_d.ap()[k * 128:(k + 1) * 128, :])
                            wps.append(w)

                # ---- projection: out[t, c] = sum_k yt_k[:, t].T @ wp_k ----
                emit_proj(yts, wps)
    nc.compile()
    return nc


def _get_nc(bias=False):
    key = ("nc", bias)
    if key not in _NC_CACHE:
        _NC_CACHE[key] = _build(bias=bias)
    return _NC_CACHE[key]


def kernel(x, w_attn, b_attn, w_proj, b_proj):
    x = np.asarray(x, dtype=np.float32)
    w_attn = np.asarray(w_attn, dtype=np.float32)
    b_attn = np.asarray(b_attn, dtype=np.float32)
    w_proj = np.asarray(w_proj, dtype=np.float32)
    b_proj = np.asarray(b_proj, dtype=np.float32)
    nc = _get_nc(bias=bool(np.any(b_attn)))
    bf = ml_dtypes.bfloat16
    in_maps = []
    for c in range(8):
        b, g = divmod(c, 2)
        xT = np.ascontiguousarray(x[b].T.astype(bf))
        s = 512 * g
        wqkv = np.ascontiguousarray(np.concatenate(
            [w_attn[:, s:s + 512],
             w_attn[:, 1024 + s:1024 + s + 512],
             w_attn[:, 2048 + s:2048 + s + 512]], axis=1).astype(bf))
        bqkv = np.ascontiguousarray(np.concatenate(
            [b_attn[s:s + 512], b_attn[1024 + s:1024 + s + 512],
             b_attn[2048 + s:2048 + s + 512]]).astype(bf))
        wp = np.ascontiguousarray(w_proj[s:s + 512, :].astype(bf))
        in_maps.append({"xT": xT, "wqkv": wqkv, "bqkv": bqkv, "wp": wp})
    globals()["_last_in_maps"] = in_maps
    res = run_bass_kernel_spmd(nc, in_maps, list(range(8)))
    out = np.empty((B, T, C), dtype=np.float32)
    for b in range(B):
        out[b] = (res.results[2 * b]["out"].astype(np.float32)
                  + res.results[2 * b + 1]["out"].astype(np.float32))
    out += b_proj
    return out
